# revision 1
# baseline (speedup 1.0000x reference)
"""GGNN message passing + bilinear readout on 8 TRN2 NeuronCores.

Problem: nn_BaselineModel_36687610642509 (gnn_message_passing).

reference:
    for 8 iters:  per_edge = einsum('sd,edh->seh', h, W_msg)
                  messages = einsum('ste,seh->th', edge, per_edge) + b_msg
                  h = GRU(h, messages)          (Wi, Wh, b_gru)
    logits = einsum('id,de,je->ij', h, A_readout, h)

Distribution (1D node parallelism, 8 cores, SENDER-sharded):
    core k owns nodes s_k = [256k, 256k+256).
    - edge shard edge[s_k, :, :] lives in SBUF for the whole kernel (bf16, 8 MiB).
    - h is sharded; each core computes per_edge for its own senders only,
      then partial messages for ALL destinations:
          msgsT_partial[d, t] = sum_e  pe_e[s_k, d]^T-contracted edge_e[s_k, t]
      One ReduceScatter(add) per iteration sums partials across cores and
      hands core k its own destination shard (dst shard == sender shard).
    - GRU update runs shard-locally; no other communication per iteration.
    - One final AllGather of h feeds the pairwise bilinear readout; each
      core emits its 256 rows of the [2048, 2048] logits.

Everything on-chip is kept in transposed [dim, node] layout so every matmul
contracts over the partition axis with zero transposes anywhere.
Matmul operands are bf16 (PSUM accumulation fp32).
"""

import sys

for _p in ("/opt/trn_rl_repo",):
    if _p not in sys.path:
        sys.path.insert(0, _p)

import numpy as np
import ml_dtypes

import concourse.bacc as bacc
import concourse.tile as tile
import concourse.mybir as mybir
from concourse import bass_utils

dt = mybir.dt
AF = mybir.ActivationFunctionType

N_CORES = 8
N = 2048          # nodes
D = 128           # embedding dim
E = 8             # edge channels
ITERS = 8
S = N // N_CORES  # 256 nodes per core
RG = [list(range(N_CORES))]


def build_nc(reps=1, wire_bf16=False):
    nc = bacc.Bacc("TRN2", target_bir_lowering=False, debug=False,
                   num_devices=N_CORES)

    edgek = nc.dram_tensor("edgek", [E * S, N], dt.bfloat16, kind="ExternalInput")
    h0t = nc.dram_tensor("h0t", [D, S], dt.bfloat16, kind="ExternalInput")
    wmsg = nc.dram_tensor("wmsg", [D, E * D], dt.bfloat16, kind="ExternalInput")
    wi = nc.dram_tensor("wi", [D, 3 * D], dt.bfloat16, kind="ExternalInput")
    wh = nc.dram_tensor("wh", [D, 3 * D], dt.bfloat16, kind="ExternalInput")
    bias = nc.dram_tensor("bias", [D, 3], dt.float32, kind="ExternalInput")
    aro = nc.dram_tensor("aro", [D, D], dt.bfloat16, kind="ExternalInput")
    out = nc.dram_tensor("out", [S, N], dt.float32, kind="ExternalOutput")

    wdt = dt.bfloat16 if wire_bf16 else dt.float32

    with tile.TileContext(nc) as tc:
        with (
            tc.tile_pool(name="const", bufs=1) as cpool,
            tc.tile_pool(name="sb", bufs=2) as spool,
            tc.tile_pool(name="stage", bufs=4) as stpool,
            tc.tile_pool(name="pe_ps", bufs=2, space="PSUM") as pe_ps,
            tc.tile_pool(name="mm_ps", bufs=3, space="PSUM") as mm_ps,
            tc.tile_pool(name="gru_ps", bufs=3, space="PSUM") as gru_ps,
            tc.tile_pool(name="dram", bufs=2, space="DRAM") as dram,
        ):
            for rep in range(reps):
                # ---- load constants (edge shard stays resident all kernel) ----
                edge_sb = {}
                for e in range(E):
                    for ss in range(2):
                        t = cpool.tile([D, N], dt.bfloat16, tag=f"edge{e}_{ss}")
                        r0 = e * S + ss * D
                        nc.sync.dma_start(t[:], edgek.ap()[r0:r0 + D, :])
                        edge_sb[(e, ss)] = t
                wmsg_sb = cpool.tile([D, E * D], dt.bfloat16, tag="wmsg")
                nc.sync.dma_start(wmsg_sb[:], wmsg.ap())
                wi_sb = cpool.tile([D, 3 * D], dt.bfloat16, tag="wi")
                nc.sync.dma_start(wi_sb[:], wi.ap())
                wh_sb = cpool.tile([D, 3 * D], dt.bfloat16, tag="wh")
                nc.sync.dma_start(wh_sb[:], wh.ap())
                bias_sb = cpool.tile([D, 3], dt.float32, tag="bias")
                nc.sync.dma_start(bias_sb[:], bias.ap())
                aro_sb = cpool.tile([D, D], dt.bfloat16, tag="aro")
                nc.sync.dma_start(aro_sb[:], aro.ap())

                hT = spool.tile([D, S], dt.bfloat16, tag="hT")
                nc.sync.dma_start(hT[:], h0t.ap())

                for it in range(ITERS):
                    # per_edge[s, (e,h)] = h_k @ [W_0 | ... | W_7]
                    pe_bf = []
                    for ss in range(2):
                        pb = spool.tile([D, E * D], dt.bfloat16, tag=f"pe{ss}")
                        for half in range(2):
                            pp = pe_ps.tile([D, 512], dt.float32, tag="pe_ps")
                            nc.tensor.matmul(
                                pp[:],
                                hT[:, ss * D:(ss + 1) * D],
                                wmsg_sb[:, half * 512:(half + 1) * 512],
                                start=True, stop=True,
                            )
                            nc.vector.tensor_copy(
                                pb[:, half * 512:(half + 1) * 512], pp[:])
                        pe_bf.append(pb)

                    # partial messages for every destination shard, then RS
                    rsin = dram.tile([N_CORES * D, S], wdt, tag="rsin")
                    for c in range(N_CORES):
                        mp = mm_ps.tile([D, S], dt.float32, tag="mm")
                        q = 0
                        for e in range(E):
                            for ss in range(2):
                                nc.tensor.matmul(
                                    mp[:],
                                    pe_bf[ss][:, e * D:(e + 1) * D],
                                    edge_sb[(e, ss)][:, c * S:(c + 1) * S],
                                    start=(q == 0), stop=(q == 15),
                                )
                                q += 1
                        st = stpool.tile([D, S], wdt, tag="mmstage")
                        nc.vector.tensor_copy(st[:], mp[:])
                        nc.sync.dma_start(rsin[c * D:(c + 1) * D, :], st[:])

                    rsout = dram.tile([D, S], wdt, tag="rsout")
                    nc.gpsimd.collective_compute(
                        "ReduceScatter", mybir.AluOpType.add,
                        replica_groups=RG,
                        ins=[rsin.opt()], outs=[rsout.opt()],
                    )
                    msgs = spool.tile([D, S], wdt, tag="msgs")
                    nc.sync.dma_start(msgs[:], rsout[:])
                    msgs_bf = spool.tile([D, S], dt.bfloat16, tag="msgsbf")
                    nc.vector.tensor_copy(msgs_bf[:], msgs[:])

                    # GRU: r/z gates via PSUM-accumulated gi+gh, biases folded
                    new_hT = spool.tile([D, S], dt.bfloat16, tag="hT")
                    gate = []
                    for g in range(2):
                        gp = gru_ps.tile([D, S], dt.float32, tag="gru")
                        nc.tensor.matmul(gp[:], wi_sb[:, g * D:(g + 1) * D],
                                         msgs_bf[:], start=True, stop=False)
                        nc.tensor.matmul(gp[:], wh_sb[:, g * D:(g + 1) * D],
                                         hT[:], start=False, stop=True)
                        gs = stpool.tile([D, S], dt.float32, tag=f"g{g}")
                        nc.scalar.activation(gs[:], gp[:], AF.Sigmoid,
                                             bias=bias_sb[:, g:g + 1])
                        gate.append(gs)
                    r_g, z_g = gate

                    inp = gru_ps.tile([D, S], dt.float32, tag="gru")
                    nc.tensor.matmul(inp[:], wi_sb[:, 2 * D:3 * D], msgs_bf[:],
                                     start=True, stop=True)
                    hnp = gru_ps.tile([D, S], dt.float32, tag="gru")
                    nc.tensor.matmul(hnp[:], wh_sb[:, 2 * D:3 * D], hT[:],
                                     start=True, stop=True)
                    t1 = stpool.tile([D, S], dt.float32, tag="t1")
                    nc.vector.tensor_mul(t1[:], r_g[:], hnp[:])
                    t2 = stpool.tile([D, S], dt.float32, tag="t2")
                    nc.vector.tensor_add(t2[:], t1[:], inp[:])
                    n_sb = stpool.tile([D, S], dt.float32, tag="n")
                    nc.scalar.activation(n_sb[:], t2[:], AF.Tanh,
                                         bias=bias_sb[:, 2:3])
                    # h_new = n + z * (h - n)
                    d1 = stpool.tile([D, S], dt.float32, tag="d1")
                    nc.vector.tensor_sub(d1[:], hT[:], n_sb[:])
                    d2 = stpool.tile([D, S], dt.float32, tag="d2")
                    nc.vector.tensor_mul(d2[:], z_g[:], d1[:])
                    nc.vector.tensor_add(new_hT[:], n_sb[:], d2[:])
                    hT = new_hT

                # ---- readout: logits rows = (h_k A) @ h^T ----
                agin = dram.tile([D, S], dt.bfloat16, tag="agin")
                nc.sync.dma_start(agin[:], hT[:])
                agout = dram.tile([N_CORES * D, S], dt.bfloat16, tag="agout")
                nc.gpsimd.collective_compute(
                    "AllGather", mybir.AluOpType.bypass,
                    replica_groups=RG,
                    ins=[agin.opt()], outs=[agout.opt()],
                )
                hTf = spool.tile([D, N], dt.bfloat16, tag="hTf")
                for j in range(N_CORES):
                    nc.sync.dma_start(hTf[:, j * S:(j + 1) * S],
                                      agout[j * D:(j + 1) * D, :])

                hap = mm_ps.tile([D, S], dt.float32, tag="mm")
                nc.tensor.matmul(hap[:], aro_sb[:], hT[:], start=True, stop=True)
                hA_bf = spool.tile([D, S], dt.bfloat16, tag="hA")
                nc.vector.tensor_copy(hA_bf[:], hap[:])

                for isub in range(2):
                    for jc in range(N_CORES):
                        lp = mm_ps.tile([D, S], dt.float32, tag="mm")
                        nc.tensor.matmul(lp[:],
                                         hA_bf[:, isub * D:(isub + 1) * D],
                                         hTf[:, jc * S:(jc + 1) * S],
                                         start=True, stop=True)
                        ost = stpool.tile([D, S], dt.float32, tag="ost")
                        nc.vector.tensor_copy(ost[:], lp[:])
                        nc.sync.dma_start(
                            out.ap()[isub * D:(isub + 1) * D,
                                     jc * S:(jc + 1) * S],
                            ost[:])

    nc.compile()
    return nc


def make_in_maps(node_embeddings, edge_embeddings, W_msg, b_msg, Wi, Wh,
                 b_gru, A_readout):
    bf16 = ml_dtypes.bfloat16
    wmsg = np.ascontiguousarray(
        W_msg.transpose(1, 0, 2).reshape(D, E * D)).astype(bf16)
    wi_b = np.ascontiguousarray(Wi).astype(bf16)
    wh_b = np.ascontiguousarray(Wh).astype(bf16)
    # messages enter the GRU only through  gi = (raw_msgs + b_msg) @ Wi + b_gru,
    # so fold b_msg into a per-gate bias (fp32, exact).
    b_eff = (b_msg.astype(np.float64) @ Wi.astype(np.float64)
             + b_gru.astype(np.float64)).astype(np.float32)
    bias = np.ascontiguousarray(b_eff.reshape(3, D).T)  # [D, 3]
    aro_b = np.ascontiguousarray(A_readout).astype(bf16)

    in_maps = []
    for k in range(N_CORES):
        sl = slice(k * S, (k + 1) * S)
        ek = np.ascontiguousarray(
            edge_embeddings[sl].transpose(2, 0, 1).reshape(E * S, N)
        ).astype(bf16)
        h0t = np.ascontiguousarray(node_embeddings[sl].T).astype(bf16)
        in_maps.append({
            "edgek": ek, "h0t": h0t, "wmsg": wmsg, "wi": wi_b, "wh": wh_b,
            "bias": bias, "aro": aro_b,
        })
    return in_maps


_cache = {}


def kernel(node_embeddings, edge_embeddings, W_msg, b_msg, Wi, Wh, b_gru,
           A_readout):
    if "nc" not in _cache:
        _cache["nc"] = build_nc(reps=1)
    nc = _cache["nc"]
    in_maps = make_in_maps(node_embeddings, edge_embeddings, W_msg, b_msg,
                           Wi, Wh, b_gru, A_readout)
    res = bass_utils.run_bass_kernel_spmd(
        nc, in_maps, core_ids=list(range(N_CORES)))
    return np.concatenate([res.results[k]["out"] for k in range(N_CORES)],
                          axis=0)


# revision 12
# speedup vs baseline: 3.6371x; 3.6371x over previous
"""GGNN message passing + bilinear readout on 8 TRN2 NeuronCores.

Problem: nn_BaselineModel_36687610642509 (gnn_message_passing).

reference:
    for 8 iters:  per_edge = einsum('sd,edh->seh', h, W_msg)
                  messages = einsum('ste,seh->th', edge, per_edge) + b_msg
                  h = GRU(h, messages)          (Wi, Wh, b_gru)
    logits = einsum('id,de,je->ij', h, A_readout, h)

Distribution (1D node parallelism, 8 cores, SENDER-sharded):
    core k owns nodes s_k = [256k, 256k+256).
    - edge shard edge[s_k, :, :] lives in SBUF for the whole kernel (bf16, 8 MiB).
    - h is sharded; each core computes per_edge for its own senders only,
      then partial messages for ALL destinations:
          msgsT_partial[d, t] = sum_e  pe_e[s_k, d]^T-contracted edge_e[s_k, t]
      One ReduceScatter(add) per iteration sums partials across cores and
      hands core k its own destination shard (dst shard == sender shard).
    - GRU update runs shard-locally; no other communication per iteration.
    - One final AllGather of h feeds the pairwise bilinear readout; each
      core emits its 256 rows of the [2048, 2048] logits.

Everything on-chip is kept in transposed [dim, node] layout so every matmul
contracts over the partition axis with zero transposes anywhere.
Matmul operands are bf16 (PSUM accumulation fp32).
"""

import sys

for _p in ("/opt/trn_rl_repo",):
    if _p not in sys.path:
        sys.path.insert(0, _p)

import numpy as np
import ml_dtypes

import concourse.bacc as bacc
import concourse.tile as tile
import concourse.mybir as mybir
from concourse import bass_utils

dt = mybir.dt
AF = mybir.ActivationFunctionType

N_CORES = 8
N = 2048          # nodes
D = 128           # embedding dim
E = 8             # edge channels
ITERS = 8
S = N // N_CORES  # 256 nodes per core
RG = [list(range(N_CORES))]


def build_nc(reps=1, wire_bf16=False, skip_coll=False, a2a=False, wide=False):
    nc = bacc.Bacc("TRN2", target_bir_lowering=False, debug=False,
                   num_devices=N_CORES)

    edgek = nc.dram_tensor("edgek", [E * S, N], dt.bfloat16, kind="ExternalInput")
    h0t = nc.dram_tensor("h0t", [D, S], dt.bfloat16, kind="ExternalInput")
    wmsg = nc.dram_tensor("wmsg", [D, E * D], dt.bfloat16, kind="ExternalInput")
    wi = nc.dram_tensor("wi", [D, 3 * D], dt.bfloat16, kind="ExternalInput")
    wh = nc.dram_tensor("wh", [D, 3 * D], dt.bfloat16, kind="ExternalInput")
    bias = nc.dram_tensor("bias", [D, 3], dt.float32, kind="ExternalInput")
    aro = nc.dram_tensor("aro", [D, D], dt.bfloat16, kind="ExternalInput")
    out = nc.dram_tensor("out", [S, N], dt.float32, kind="ExternalOutput")

    wdt = dt.bfloat16 if wire_bf16 else dt.float32

    with tile.TileContext(nc) as tc:
        with (
            tc.tile_pool(name="const", bufs=1) as cpool,
            tc.tile_pool(name="sb", bufs=2) as spool,
            tc.tile_pool(name="stage", bufs=4) as stpool,
            tc.tile_pool(name="pe_ps", bufs=2, space="PSUM") as pe_ps,
            tc.tile_pool(name="mm_ps", bufs=3, space="PSUM") as mm_ps,
            tc.tile_pool(name="gru_ps", bufs=3, space="PSUM") as gru_ps,
            tc.tile_pool(name="dram", bufs=2, space="DRAM") as dram,
        ):
            for rep in range(reps):
                # ---- load constants (edge shard stays resident all kernel) ----
                edge_sb = {}
                for e in range(E):
                    for ss in range(2):
                        t = cpool.tile([D, N], dt.bfloat16, tag=f"edge{e}_{ss}")
                        r0 = e * S + ss * D
                        nc.sync.dma_start(t[:], edgek.ap()[r0:r0 + D, :])
                        edge_sb[(e, ss)] = t
                wmsg_sb = cpool.tile([D, E * D], dt.bfloat16, tag="wmsg")
                nc.sync.dma_start(wmsg_sb[:], wmsg.ap())
                wi_sb = cpool.tile([D, 3 * D], dt.bfloat16, tag="wi")
                nc.sync.dma_start(wi_sb[:], wi.ap())
                wh_sb = cpool.tile([D, 3 * D], dt.bfloat16, tag="wh")
                nc.sync.dma_start(wh_sb[:], wh.ap())
                bias_sb = cpool.tile([D, 3], dt.float32, tag="bias")
                nc.sync.dma_start(bias_sb[:], bias.ap())
                aro_sb = cpool.tile([D, D], dt.bfloat16, tag="aro")
                nc.sync.dma_start(aro_sb[:], aro.ap())

                hT = spool.tile([D, S], dt.bfloat16, tag="hT")
                nc.sync.dma_start(hT[:], h0t.ap())

                for it in range(ITERS):
                    # per_edge[s, (e,h)] = h_k @ [W_0 | ... | W_7]
                    pe_bf = []
                    for ss in range(2):
                        pb = spool.tile([D, E * D], dt.bfloat16, tag=f"pe{ss}")
                        for half in range(2):
                            pp = pe_ps.tile([D, 512], dt.float32, tag="pe_ps")
                            nc.tensor.matmul(
                                pp[:],
                                hT[:, ss * D:(ss + 1) * D],
                                wmsg_sb[:, half * 512:(half + 1) * 512],
                                start=True, stop=True,
                            )
                            nc.vector.tensor_copy(
                                pb[:, half * 512:(half + 1) * 512], pp[:])
                        pe_bf.append(pb)

                    # partial messages for every destination shard, then RS
                    rsin = dram.tile([N_CORES * D, S], wdt, tag="rsin")
                    CW = 2 * S if wide else S  # big-mm moving width
                    for c in range(N * 1 // CW):
                        mp = mm_ps.tile([D, CW], dt.float32, tag="mm")
                        q = 0
                        for e in range(E):
                            for ss in range(2):
                                nc.tensor.matmul(
                                    mp[:],
                                    pe_bf[ss][:, e * D:(e + 1) * D],
                                    edge_sb[(e, ss)][:, c * CW:(c + 1) * CW],
                                    start=(q == 0), stop=(q == 15),
                                )
                                q += 1
                        st = stpool.tile([D, CW], wdt, tag="mmstage")
                        nc.vector.tensor_copy(st[:], mp[:])
                        for b in range(CW // S):
                            j = c * (CW // S) + b
                            nc.sync.dma_start(rsin[j * D:(j + 1) * D, :],
                                              st[:, b * S:(b + 1) * S])

                    if a2a:
                        a2out = dram.tile([N_CORES * D, S], wdt, tag="a2out")
                        if not skip_coll:
                            nc.gpsimd.collective_compute(
                                "AllToAll", mybir.AluOpType.bypass,
                                replica_groups=RG,
                                ins=[rsin.opt()], outs=[a2out.opt()],
                            )
                        else:
                            a2out = rsin
                        # local 8-way sum of the received partials
                        parts = spool.tile([D, N_CORES * S], wdt, tag="parts")
                        for j in range(N_CORES):
                            nc.sync.dma_start(parts[:, j * S:(j + 1) * S],
                                              a2out[j * D:(j + 1) * D, :])
                        acc = []
                        for l in range(2):
                            acc_t = spool.tile([D, S], dt.float32, tag=f"acc{l}")
                            acc.append(acc_t)
                        nc.vector.tensor_add(acc[0][:], parts[:, 0:S],
                                             parts[:, S:2 * S])
                        for j in range(2, N_CORES - 1):
                            nc.vector.tensor_add(acc[j % 2][:],
                                                 acc[(j + 1) % 2][:],
                                                 parts[:, j * S:(j + 1) * S])
                        msgs_bf = spool.tile([D, S], dt.bfloat16, tag="msgsbf")
                        nc.vector.tensor_add(msgs_bf[:], acc[1][:],
                                             parts[:, 7 * S:8 * S])
                    else:
                        rsout = dram.tile([D, S], wdt, tag="rsout")
                        if not skip_coll:
                            nc.gpsimd.collective_compute(
                                "ReduceScatter", mybir.AluOpType.add,
                                replica_groups=RG,
                                ins=[rsin.opt()], outs=[rsout.opt()],
                            )
                            msgs_src = rsout
                        else:
                            msgs_src = rsin[0:D, :]
                        msgs = spool.tile([D, S], wdt, tag="msgs")
                        nc.sync.dma_start(msgs[:], msgs_src[:] if msgs_src is rsout else msgs_src)
                        if wire_bf16:
                            msgs_bf = msgs
                        else:
                            msgs_bf = spool.tile([D, S], dt.bfloat16, tag="msgsbf")
                            nc.vector.tensor_copy(msgs_bf[:], msgs[:])

                    # GRU: r/z gates via PSUM-accumulated gi+gh, biases folded
                    new_hT = spool.tile([D, S], dt.bfloat16, tag="hT")
                    gate = []
                    for g in range(2):
                        gp = gru_ps.tile([D, S], dt.float32, tag="gru")
                        nc.tensor.matmul(gp[:], wi_sb[:, g * D:(g + 1) * D],
                                         msgs_bf[:], start=True, stop=False)
                        nc.tensor.matmul(gp[:], wh_sb[:, g * D:(g + 1) * D],
                                         hT[:], start=False, stop=True)
                        gs = stpool.tile([D, S], dt.float32, tag=f"g{g}")
                        nc.scalar.activation(gs[:], gp[:], AF.Sigmoid,
                                             bias=bias_sb[:, g:g + 1])
                        gate.append(gs)
                    r_g, z_g = gate

                    inp = gru_ps.tile([D, S], dt.float32, tag="gru")
                    nc.tensor.matmul(inp[:], wi_sb[:, 2 * D:3 * D], msgs_bf[:],
                                     start=True, stop=True)
                    hnp = gru_ps.tile([D, S], dt.float32, tag="gru")
                    nc.tensor.matmul(hnp[:], wh_sb[:, 2 * D:3 * D], hT[:],
                                     start=True, stop=True)
                    t1 = stpool.tile([D, S], dt.float32, tag="t1")
                    nc.vector.tensor_mul(t1[:], r_g[:], hnp[:])
                    t2 = stpool.tile([D, S], dt.float32, tag="t2")
                    nc.vector.tensor_add(t2[:], t1[:], inp[:])
                    n_sb = stpool.tile([D, S], dt.float32, tag="n")
                    nc.scalar.activation(n_sb[:], t2[:], AF.Tanh,
                                         bias=bias_sb[:, 2:3])
                    # h_new = n + z * (h - n)
                    d1 = stpool.tile([D, S], dt.float32, tag="d1")
                    nc.vector.tensor_sub(d1[:], hT[:], n_sb[:])
                    d2 = stpool.tile([D, S], dt.float32, tag="d2")
                    nc.vector.tensor_mul(d2[:], z_g[:], d1[:])
                    nc.vector.tensor_add(new_hT[:], n_sb[:], d2[:])
                    hT = new_hT

                # ---- readout: logits rows = (h_k A) @ h^T ----
                agin = dram.tile([D, S], dt.bfloat16, tag="agin")
                nc.sync.dma_start(agin[:], hT[:])
                agout = dram.tile([N_CORES * D, S], dt.bfloat16, tag="agout")
                if not skip_coll:
                    nc.gpsimd.collective_compute(
                        "AllGather", mybir.AluOpType.bypass,
                        replica_groups=RG,
                        ins=[agin.opt()], outs=[agout.opt()],
                    )
                hTf = spool.tile([D, N], dt.bfloat16, tag="hTf")
                for j in range(N_CORES):
                    src = agout[j * D:(j + 1) * D, :] if not skip_coll else agin[:]
                    nc.sync.dma_start(hTf[:, j * S:(j + 1) * S], src)

                hap = mm_ps.tile([D, S], dt.float32, tag="mm")
                nc.tensor.matmul(hap[:], aro_sb[:], hT[:], start=True, stop=True)
                hA_bf = spool.tile([D, S], dt.bfloat16, tag="hA")
                nc.vector.tensor_copy(hA_bf[:], hap[:])

                for isub in range(2):
                    for jc in range(N_CORES):
                        lp = mm_ps.tile([D, S], dt.float32, tag="mm")
                        nc.tensor.matmul(lp[:],
                                         hA_bf[:, isub * D:(isub + 1) * D],
                                         hTf[:, jc * S:(jc + 1) * S],
                                         start=True, stop=True)
                        ost = stpool.tile([D, S], dt.float32, tag="ost")
                        nc.vector.tensor_copy(ost[:], lp[:])
                        nc.sync.dma_start(
                            out.ap()[isub * D:(isub + 1) * D,
                                     jc * S:(jc + 1) * S],
                            ost[:])

    nc.compile()
    return nc


def make_in_maps(node_embeddings, edge_embeddings, W_msg, b_msg, Wi, Wh,
                 b_gru, A_readout):
    bf16 = ml_dtypes.bfloat16
    wmsg = np.ascontiguousarray(
        W_msg.transpose(1, 0, 2).reshape(D, E * D)).astype(bf16)
    wi_b = np.ascontiguousarray(Wi).astype(bf16)
    wh_b = np.ascontiguousarray(Wh).astype(bf16)
    # messages enter the GRU only through  gi = (raw_msgs + b_msg) @ Wi + b_gru,
    # so fold b_msg into a per-gate bias (fp32, exact).
    b_eff = (b_msg.astype(np.float64) @ Wi.astype(np.float64)
             + b_gru.astype(np.float64)).astype(np.float32)
    bias = np.ascontiguousarray(b_eff.reshape(3, D).T)  # [D, 3]
    aro_b = np.ascontiguousarray(A_readout).astype(bf16)

    in_maps = []
    for k in range(N_CORES):
        sl = slice(k * S, (k + 1) * S)
        ek = np.ascontiguousarray(
            edge_embeddings[sl].transpose(2, 0, 1).reshape(E * S, N)
        ).astype(bf16)
        h0t = np.ascontiguousarray(node_embeddings[sl].T).astype(bf16)
        in_maps.append({
            "edgek": ek, "h0t": h0t, "wmsg": wmsg, "wi": wi_b, "wh": wh_b,
            "bias": bias, "aro": aro_b,
        })
    return in_maps


_cache = {}


def kernel(node_embeddings, edge_embeddings, W_msg, b_msg, Wi, Wh, b_gru,
           A_readout):
    if "nc" not in _cache:
        _cache["nc"] = build_nc(reps=1, wire_bf16=True, wide=True)
    nc = _cache["nc"]
    in_maps = make_in_maps(node_embeddings, edge_embeddings, W_msg, b_msg,
                           Wi, Wh, b_gru, A_readout)
    res = bass_utils.run_bass_kernel_spmd(
        nc, in_maps, core_ids=list(range(N_CORES)))
    return np.concatenate([res.results[k]["out"] for k in range(N_CORES)],
                          axis=0)
